# revision 6
# baseline (speedup 1.0000x reference)
"""Trainium2 Bass kernel for nn_MixedPatchEmbed.

Mixed 16/32 patch embedding with entropy-driven token selection and ragged
compaction, data-parallel over batch across 8 NeuronCores.

Per-core device strategy (16 images/core):
  - x loaded once per image as row-bands: SBUF tile [96 partitions=(i32,c),
    free=(img, gy32, w)] -- contiguous 896B row segments, no im2col in HBM.
  - conv-as-matmul with contraction over (c, i, j) accumulated in PSUM over
    the patch column offset j, which lives in the free-dim ACCESS-PATTERN
    OFFSET of the moving operand (rows are already on partitions).
  - 16-path raises K-utilization to 96 by pairing j with j+8 via a
    DVE-built shifted row copy (partitions 48..95 = rows shifted 8 elems).
  - weights are host-repacked to match the partition layout; pos embeds
    (+bias) host-folded and added during PSUM evacuation.
  - PSUM [D-tile, tokens] -> PE transpose -> token-major [tokens, 768]
    -> indirect DMA scatter straight to the compacted output rows
    (host-computed int32 ranks; dropped tokens go to a trash row).
  - cls token rows / seqlens / cls_token_indices are pure mask arithmetic,
    filled on host during unshard.
"""

import numpy as np

import concourse.bass as bass
import concourse.mybir as mybir
import concourse.tile as tile
from concourse.bass_utils import run_bass_kernel_spmd
from concourse.masks import make_identity
from concourse.vector_clock import ScopedClock

F32 = mybir.dt.float32
I32 = mybir.dt.int32

B, C, IMG, D = 128, 3, 224, 768
P16, P32 = 16, 32
G16, G32 = 14, 7
N16, N32 = 196, 49
THRESHOLD = 7.0

NCORES = 8
PERCORE = B // NCORES            # 16 images per core
NTOK = PERCORE * (N16 + N32)     # 3920 conv tokens per core
TRASH = PERCORE * (1 + N16 + N32)  # 3936: trash row index
OUT_ROWS = TRASH + 1             # 3937

GROUPS = 4                       # 4-image groups
GI = PERCORE // GROUPS           # 4 images per group
ROWFREE = G32 * IMG              # 1568 floats per image per partition
XW = GI * ROWFREE                # 6272 free dim of x tile
M16 = GI * G32 * G16             # 392 tokens per (group, parity)
M32 = GI * N32                   # 196 tokens per group (32-path)
NSCAT = GROUPS * (2 * 4 + 2)     # 40 scatter tiles

_CHUNKS16 = [(0, 128), (128, 128), (256, 128), (384, 8)]
_CHUNKS32 = [(0, 128), (128, 68)]


# ---------------------------------------------------------------------------
# compat patches: this walrus build accepts only ONE sync-wait / instruction
# ---------------------------------------------------------------------------

def _patched_drain_and_barrier(self, tick_clock, wait_clock):
    nc = self.nc
    drain_inst = nc.sync.drain()
    wait_clock.add_sem_waits(
        drain_inst.ins, ScopedClock({None: tick_clock.global_clock})
    )
    si = drain_inst.ins.sync_info
    waits = list(si.on_wait or [])
    if len(waits) > 1:
        si.on_wait = waits[:1]
        for w in waits[1:]:
            nop = nc.sync.nop(nofuse=True, hint="drain_wait_split")
            nsi = nop.ins.sync_info
            if nsi is None:
                nsi = mybir.SyncInfo(on_update=[], on_wait=[])
                nop.ins.sync_info = nsi
            nsi.on_wait = [w]
    nc.all_engine_barrier()
    assert self.sems is not None
    popped = nc._tile_sem_poison_stack.pop()
    assert popped is self._sem_poison
    nc.clear_and_free_semaphores(list(self.sems.allocated().values()))
    nc.all_engine_barrier()


tile.TileContext._drain_and_barrier = _patched_drain_and_barrier


def _split_multi_waits(nc):
    for _name, bbw in nc.bb_map.items():
        bb = bbw.bb if hasattr(bbw, "bb") else bbw
        out = []
        changed = False
        for inst in bb.instructions:
            si = inst.sync_info
            waits = list(si.on_wait) if si is not None and si.on_wait else []
            if len(waits) > 1:
                changed = True
                for w in waits[:-1]:
                    nop = mybir.InstNoOp(
                        name=nc.get_next_instruction_name(),
                        ins=[],
                        outs=[],
                        engine=inst.engine,
                        sync_info=mybir.SyncInfo(on_update=[], on_wait=[w]),
                    )
                    nc.register_instruction(nop)
                    out.append(nop)
                si.on_wait = [waits[-1]]
            out.append(inst)
        if changed:
            bb.instructions[:] = out


# ---------------------------------------------------------------------------
# host-side prep
# ---------------------------------------------------------------------------

def _masks(entropy_maps):
    mask32 = entropy_maps < THRESHOLD                      # [B,7,7]
    mask16 = np.repeat(np.repeat(~mask32, 2, axis=1), 2, axis=2)
    return mask32, mask16


def _pack_weights(w16, w32):
    # partition layout: p = 64*half + (i16*3 + c); rows 48..63 / 112..127 unused
    r48 = np.arange(48)
    i16, c16 = r48 // 3, r48 % 3
    W16d = np.zeros((8, 112, D), np.float32)
    for j in range(8):
        W16d[j, :48] = w16[:, c16, i16, j].T
        W16d[j, 64:] = w16[:, c16, i16, j + 8].T
    W32d = np.zeros((32, 112, D), np.float32)
    for j in range(32):
        W32d[j, :48] = w32[:, c16, i16, j].T          # i32 = i16 (0..15)
        W32d[j, 64:] = w32[:, c16, i16 + 16, j].T     # i32 = 16..31
    return W16d, W32d


def _pack_pos(pos16, b16, pos32, b32):
    p16 = (np.asarray(pos16[0]) + np.asarray(b16)[None, :]).astype(np.float32)
    # token16 index gy16*14+gx with gy16 = 2*gy32 + pi
    arr = p16.reshape(G32, 2, G16, D)            # [gy32, pi, gx, D]
    pos16d = arr.transpose(3, 1, 0, 2).reshape(D, 2, 98)
    pos16d = np.ascontiguousarray(pos16d).reshape(6, 128, 2, 98)
    p32 = (np.asarray(pos32[0]) + np.asarray(b32)[None, :]).astype(np.float32)
    pos32d = np.ascontiguousarray(p32.T).reshape(6, 128, N32)
    return pos16d, pos32d


def _scatter_indices(flat_keep, rank):
    """Destination row for every token in device stream order.

    flat_keep/rank: [PERCORE*246] bool / running-rank over this core's
    reference-ordered token list (cls + 196 + 49 per image).
    Returns [128, NSCAT] int32 (host-pretransposed for the SBUF layout).
    """
    idx_tiles = np.full((NSCAT, 128), TRASH, np.int32)
    t = 0

    def emit16(g, pi, t):
        dests = np.full(M16, TRASH, np.int64)
        k = 0
        for ig in range(GI):
            img = GI * g + ig
            for gy32 in range(G32):
                gy16 = 2 * gy32 + pi
                base = img * 246 + 1 + gy16 * G16
                for gx in range(G16):
                    fi = base + gx
                    dests[k] = rank[fi] if flat_keep[fi] else TRASH
                    k += 1
        for c0, cw in _CHUNKS16:
            idx_tiles[t, :cw] = dests[c0:c0 + cw]
            t += 1
        return t

    def emit32(g, t):
        dests = np.full(M32, TRASH, np.int64)
        k = 0
        for ig in range(GI):
            img = GI * g + ig
            base = img * 246 + 1 + N16
            for mm in range(N32):
                fi = base + mm
                dests[k] = rank[fi] if flat_keep[fi] else TRASH
                k += 1
        for c0, cw in _CHUNKS32:
            idx_tiles[t, :cw] = dests[c0:c0 + cw]
            t += 1
        return t

    for g in range(GROUPS):
        # must match device emission order: pi0 16-path, 32-path, pi1 16-path
        t = emit16(g, 0, t)
        t = emit32(g, t)
        t = emit16(g, 1, t)
    assert t == NSCAT
    return np.ascontiguousarray(idx_tiles.T)


# ---------------------------------------------------------------------------
# device program
# ---------------------------------------------------------------------------

def _build_program():
    nc = bass.Bass()
    xin = nc.declare_dram_parameter("x", [PERCORE, C, IMG, IMG], F32, isOutput=False)
    w16in = nc.declare_dram_parameter("w16d", [8, 112, D], F32, isOutput=False)
    w32in = nc.declare_dram_parameter("w32d", [32, 112, D], F32, isOutput=False)
    p16in = nc.declare_dram_parameter("pos16d", [6, 128, 2, 98], F32, isOutput=False)
    p32in = nc.declare_dram_parameter("pos32d", [6, 128, N32], F32, isOutput=False)
    idxin = nc.declare_dram_parameter("idxd", [128, NSCAT], I32, isOutput=False)
    outb = nc.declare_dram_parameter("outb", [OUT_ROWS, D], F32, isOutput=True)

    with tile.TileContext(nc) as tc:
        with (
            tc.tile_pool(name="const", bufs=1) as cpool,
            tc.tile_pool(name="xp", bufs=1) as xpool,
            tc.tile_pool(name="x2p", bufs=1) as x2pool,
            tc.tile_pool(name="evp", bufs=1) as evpool,
            tc.tile_pool(name="tmp", bufs=2) as tmpool,
            tc.tile_pool(name="mmps", bufs=2, space="PSUM") as mmpool,
            tc.tile_pool(name="trps", bufs=3, space="PSUM") as trpool,
        ):
            w16sb = cpool.tile([112, 8 * D], F32, tag="w16")
            w32sb = cpool.tile([112, 32 * D], F32, tag="w32")
            # pos16 | pos32 | identity | idx (int32 view), one padded slot
            csb = cpool.tile([128, 1176 + 294 + 128 + NSCAT], F32, tag="cc")
            pos16sb = csb[:, 0:1176]
            pos32sb = csb[:, 1176:1470]
            ident = csb[:, 1470:1598]
            idxsb = csb[:, 1598:1598 + NSCAT].bitcast(I32)
            make_identity(nc, ident)

            nc.sync.dma_start(
                out=w16sb[:],
                in_=w16in[:].rearrange("j r d -> r j d"),
            )
            nc.sync.dma_start(
                out=w32sb[:],
                in_=w32in[:].rearrange("j r d -> r j d"),
            )
            nc.sync.dma_start(
                out=pos16sb,
                in_=p16in[:].rearrange("t p pi m -> p t pi m"),
            )
            nc.sync.dma_start(
                out=pos32sb,
                in_=p32in[:].rearrange("t p m -> p t m"),
            )
            nc.sync.dma_start(out=idxsb, in_=idxin[:])

            def scatter(tm_ap, cw, stream_tile):
                nc.gpsimd.indirect_dma_start(
                    out=outb[:],
                    out_offset=bass.IndirectOffsetOnAxis(
                        ap=idxsb[0:cw, stream_tile:stream_tile + 1], axis=0
                    ),
                    in_=tm_ap,
                    in_offset=None,
                )

            def transpose_and_scatter(ev6, mtot, chunks, stream_tile):
                for c0, cw in chunks:
                    tm = tmpool.tile([128, 6 * 128], F32, tag="tm")
                    for dt in range(6):
                        tr = trpool.tile([128, 128], F32, tag="tr")
                        nc.tensor.transpose(
                            out=tr[0:cw, :],
                            in_=ev6[:, dt * M16 + c0: dt * M16 + c0 + cw],
                            identity=ident,
                        )
                        nc.scalar.copy(
                            out=tm[0:cw, dt * 128:(dt + 1) * 128],
                            in_=tr[0:cw, :],
                        )
                    scatter(tm[0:cw, :], cw, stream_tile)
                    stream_tile += 1
                return stream_tile

            def path16(xsb, pi, stream_tile):
                x2 = x2pool.tile([128, XW], F32, tag="x2")
                nc.vector.tensor_copy(
                    out=x2[0:48, :], in_=xsb[64 * pi:64 * pi + 48, :]
                )
                nc.vector.tensor_copy(
                    out=x2[64:112, 0:XW - 8],
                    in_=xsb[64 * pi:64 * pi + 48, 8:XW],
                )
                x2r = x2[0:112].rearrange(
                    "p (ig gy gx j) -> p ig gy gx j", ig=GI, gy=G32, gx=G16
                )
                ev6 = evpool.tile([128, 6 * M16], F32, tag="ev")
                for dt in range(6):
                    ps = mmpool.tile([128, M16], F32, tag="mm")
                    for j in range(8):
                        nc.tensor.matmul(
                            out=ps[:],
                            lhsT=w16sb[:, j * D + dt * 128: j * D + dt * 128 + 128],
                            rhs=x2r[:, :, :, :, j],
                            start=(j == 0),
                            stop=(j == 7),
                        )
                    s0 = (dt * 2 + pi) * 98
                    pslice = pos16sb[:, s0:s0 + 98]
                    pbc = bass.AP(
                        pslice.tensor, pslice.offset,
                        [pslice.ap[0], [0, GI], [1, 98]],
                    )
                    nc.vector.tensor_tensor(
                        out=ev6[:, dt * M16:(dt + 1) * M16].rearrange(
                            "p (ig m) -> p ig m", ig=GI),
                        in0=ps[:].rearrange("p (ig m) -> p ig m", ig=GI),
                        in1=pbc,
                        op=mybir.AluOpType.add,
                    )
                return transpose_and_scatter(ev6, M16, _CHUNKS16, stream_tile)

            def path32(xsb, stream_tile):
                xr = xsb[0:112].rearrange(
                    "p (ig gy gx j) -> p ig gy gx j", ig=GI, gy=G32, gx=G32
                )
                ev6 = evpool.tile([128, 6 * M16], F32, tag="ev")
                for dt in range(6):
                    ps = mmpool.tile([128, M16], F32, tag="mm")
                    for j in range(32):
                        nc.tensor.matmul(
                            out=ps[:, 0:M32],
                            lhsT=w32sb[:, j * D + dt * 128: j * D + dt * 128 + 128],
                            rhs=xr[:, :, :, :, j],
                            start=(j == 0),
                            stop=(j == 31),
                        )
                    pslice = pos32sb[:, dt * N32:(dt + 1) * N32]
                    pbc = bass.AP(
                        pslice.tensor, pslice.offset,
                        [pslice.ap[0], [0, GI], [1, N32]],
                    )
                    nc.vector.tensor_tensor(
                        out=ev6[:, dt * M16: dt * M16 + M32].rearrange(
                            "p (ig m) -> p ig m", ig=GI),
                        in0=ps[:, 0:M32].rearrange("p (ig m) -> p ig m", ig=GI),
                        in1=pbc,
                        op=mybir.AluOpType.add,
                    )
                return transpose_and_scatter(ev6, M32, _CHUNKS32, stream_tile)

            # one-time zero of the never-written partition bands (48:64,
            # 112:128) of the single x / x2 slots so garbage*0 can't make NaN
            xz = xpool.tile([128, XW], F32, tag="x")
            x2z = x2pool.tile([128, XW], F32, tag="x2")
            for z in (xz, x2z):
                nc.gpsimd.memset(z[32:64, :], 0.0)
                nc.gpsimd.memset(z[96:128, :], 0.0)

            stream_tile = 0
            for g in range(GROUPS):
                xsb = xpool.tile([128, XW], F32, tag="x")
                for ig in range(GI):
                    img = GI * g + ig
                    imgr = xin[:][img].rearrange(
                        "c (gy pi i) w -> pi i c gy w", pi=2, i=16
                    )
                    for pi in range(2):
                        nc.sync.dma_start(
                            out=xsb[64 * pi:64 * pi + 48,
                                    ig * ROWFREE:(ig + 1) * ROWFREE],
                            in_=imgr[pi],
                        )
                # order: pi0 16-path, 32-path, pi1 16-path — so the pi1 x2
                # copy (WAR on pi0 matmuls) overlaps the 32-path matmuls,
                # and the next group's x DMA overlaps pi1 matmuls.
                stream_tile = path16(xsb, 0, stream_tile)
                stream_tile = path32(xsb, stream_tile)
                stream_tile = path16(xsb, 1, stream_tile)
            assert stream_tile == NSCAT

    _split_multi_waits(nc)
    return nc


_PROGRAM_CACHE = {}


def _get_program():
    if "nc" not in _PROGRAM_CACHE:
        _PROGRAM_CACHE["nc"] = _build_program()
    return _PROGRAM_CACHE["nc"]


# ---------------------------------------------------------------------------
# public entry point
# ---------------------------------------------------------------------------

def kernel(x, entropy_maps, w16, b16, w32, b32, pos16, pos32, cls_token,
           _want_timing=False):
    x = np.ascontiguousarray(np.asarray(x), dtype=np.float32)
    entropy_maps = np.asarray(entropy_maps)
    w16 = np.asarray(w16, dtype=np.float32)
    b16 = np.asarray(b16, dtype=np.float32)
    w32 = np.asarray(w32, dtype=np.float32)
    b32 = np.asarray(b32, dtype=np.float32)
    pos16 = np.asarray(pos16, dtype=np.float32)
    pos32 = np.asarray(pos32, dtype=np.float32)
    cls_token = np.asarray(cls_token, dtype=np.float32)

    mask32, mask16 = _masks(np.asarray(entropy_maps, dtype=np.float32))
    m_all = np.concatenate(
        [np.ones((B, 1), bool), mask16.reshape(B, N16), mask32.reshape(B, N32)],
        axis=1,
    )  # [B, 246]
    seqlens = m_all.sum(axis=1).astype(np.int32)
    cls_token_indices = np.concatenate(
        [np.zeros(1, np.int64), np.cumsum(seqlens)[:-1]]
    ).astype(np.int32)

    W16d, W32d = _pack_weights(w16, w32)
    pos16d, pos32d = _pack_pos(pos16, b16, pos32, b32)

    in_maps = []
    counts = []
    for core in range(NCORES):
        sub = m_all[PERCORE * core: PERCORE * (core + 1)]
        flat = sub.reshape(-1)
        rank = np.cumsum(flat) - 1
        counts.append(int(flat.sum()))
        idxd = _scatter_indices(flat, rank)
        in_maps.append(dict(
            x=x[PERCORE * core: PERCORE * (core + 1)],
            w16d=W16d, w32d=W32d, pos16d=pos16d, pos32d=pos32d, idxd=idxd,
        ))

    nc = _get_program()
    kres = run_bass_kernel_spmd(
        nc, in_maps, list(range(NCORES)), trace=bool(_want_timing)
    )
    results = kres.results

    ntotal = int(seqlens.sum())
    out = np.empty((ntotal, D), np.float32)
    off = 0
    for core in range(NCORES):
        n = counts[core]
        out[off:off + n] = results[core]["outb"][:n]
        off += n
    assert off == ntotal
    out[cls_token_indices.astype(np.int64)] = cls_token[0, 0]

    ret = (out[None], seqlens, cls_token_indices)
    if _want_timing:
        return ret, kres
    return ret


# revision 7
# speedup vs baseline: 4.0814x; 4.0814x over previous
"""Trainium2 Bass kernel for nn_MixedPatchEmbed.

Mixed 16/32 patch embedding with entropy-driven token selection and ragged
compaction, data-parallel over batch across 8 NeuronCores.

Per-core device strategy (16 images/core):
  - x loaded once per image as row-bands: SBUF tile [96 partitions=(i32,c),
    free=(img, gy32, w)] -- contiguous 896B row segments, no im2col in HBM.
  - conv-as-matmul with contraction over (c, i, j) accumulated in PSUM over
    the patch column offset j, which lives in the free-dim ACCESS-PATTERN
    OFFSET of the moving operand (rows are already on partitions).
  - 16-path raises K-utilization to 96 by pairing j with j+8 via a
    DVE-built shifted row copy (partitions 48..95 = rows shifted 8 elems).
  - weights are host-repacked to match the partition layout; pos embeds
    (+bias) host-folded and added during PSUM evacuation.
  - PSUM [D-tile, tokens] -> PE transpose -> token-major [tokens, 768]
    -> indirect DMA scatter straight to the compacted output rows
    (host-computed int32 ranks; dropped tokens go to a trash row).
  - cls token rows / seqlens / cls_token_indices are pure mask arithmetic,
    filled on host during unshard.
"""

import numpy as np

import concourse.bass as bass
import concourse.mybir as mybir
import concourse.tile as tile
from concourse.bass_utils import run_bass_kernel_spmd
from concourse.masks import make_identity
from concourse.vector_clock import ScopedClock

F32 = mybir.dt.float32
I32 = mybir.dt.int32

B, C, IMG, D = 128, 3, 224, 768
P16, P32 = 16, 32
G16, G32 = 14, 7
N16, N32 = 196, 49
THRESHOLD = 7.0

NCORES = 8
PERCORE = B // NCORES            # 16 images per core
NTOK = PERCORE * (N16 + N32)     # 3920 conv tokens per core
TRASH = PERCORE * (1 + N16 + N32)  # 3936: trash row index
OUT_ROWS = TRASH + 1             # 3937

GROUPS = 4                       # 4-image groups
GI = PERCORE // GROUPS           # 4 images per group
ROWFREE = G32 * IMG              # 1568 floats per image per partition
XW = GI * ROWFREE                # 6272 free dim of x tile
M16 = GI * G32 * G16             # 392 tokens per (group, parity)
M32 = GI * N32                   # 196 tokens per group (32-path)
NSCAT = GROUPS * (2 * 4 + 2)     # 40 scatter tiles

_CHUNKS16 = [(0, 128), (128, 128), (256, 128), (384, 8)]
_CHUNKS32 = [(0, 128), (128, 68)]


# ---------------------------------------------------------------------------
# compat patches: this walrus build accepts only ONE sync-wait / instruction
# ---------------------------------------------------------------------------

def _patched_drain_and_barrier(self, tick_clock, wait_clock):
    nc = self.nc
    drain_inst = nc.sync.drain()
    wait_clock.add_sem_waits(
        drain_inst.ins, ScopedClock({None: tick_clock.global_clock})
    )
    si = drain_inst.ins.sync_info
    waits = list(si.on_wait or [])
    if len(waits) > 1:
        si.on_wait = waits[:1]
        for w in waits[1:]:
            nop = nc.sync.nop(nofuse=True, hint="drain_wait_split")
            nsi = nop.ins.sync_info
            if nsi is None:
                nsi = mybir.SyncInfo(on_update=[], on_wait=[])
                nop.ins.sync_info = nsi
            nsi.on_wait = [w]
    nc.all_engine_barrier()
    assert self.sems is not None
    popped = nc._tile_sem_poison_stack.pop()
    assert popped is self._sem_poison
    nc.clear_and_free_semaphores(list(self.sems.allocated().values()))
    nc.all_engine_barrier()


tile.TileContext._drain_and_barrier = _patched_drain_and_barrier


def _split_multi_waits(nc):
    for _name, bbw in nc.bb_map.items():
        bb = bbw.bb if hasattr(bbw, "bb") else bbw
        out = []
        changed = False
        for inst in bb.instructions:
            si = inst.sync_info
            waits = list(si.on_wait) if si is not None and si.on_wait else []
            if len(waits) > 1:
                changed = True
                for w in waits[:-1]:
                    nop = mybir.InstNoOp(
                        name=nc.get_next_instruction_name(),
                        ins=[],
                        outs=[],
                        engine=inst.engine,
                        sync_info=mybir.SyncInfo(on_update=[], on_wait=[w]),
                    )
                    nc.register_instruction(nop)
                    out.append(nop)
                si.on_wait = [waits[-1]]
            out.append(inst)
        if changed:
            bb.instructions[:] = out


# ---------------------------------------------------------------------------
# host-side prep
# ---------------------------------------------------------------------------

def _masks(entropy_maps):
    mask32 = entropy_maps < THRESHOLD                      # [B,7,7]
    mask16 = np.repeat(np.repeat(~mask32, 2, axis=1), 2, axis=2)
    return mask32, mask16


def _pack_weights(w16, w32):
    # partition layout: p = 64*half + (i16*3 + c); rows 48..63 / 112..127 unused
    r48 = np.arange(48)
    i16, c16 = r48 // 3, r48 % 3
    W16d = np.zeros((8, 112, D), np.float32)
    for j in range(8):
        W16d[j, :48] = w16[:, c16, i16, j].T
        W16d[j, 64:] = w16[:, c16, i16, j + 8].T
    W32d = np.zeros((32, 112, D), np.float32)
    for j in range(32):
        W32d[j, :48] = w32[:, c16, i16, j].T          # i32 = i16 (0..15)
        W32d[j, 64:] = w32[:, c16, i16 + 16, j].T     # i32 = 16..31
    return W16d, W32d


def _pack_pos(pos16, b16, pos32, b32):
    p16 = (np.asarray(pos16[0]) + np.asarray(b16)[None, :]).astype(np.float32)
    # token16 index gy16*14+gx with gy16 = 2*gy32 + pi
    arr = p16.reshape(G32, 2, G16, D)            # [gy32, pi, gx, D]
    pos16d = arr.transpose(3, 1, 0, 2).reshape(D, 2, 98)
    pos16d = np.ascontiguousarray(pos16d).reshape(6, 128, 2, 98)
    p32 = (np.asarray(pos32[0]) + np.asarray(b32)[None, :]).astype(np.float32)
    pos32d = np.ascontiguousarray(p32.T).reshape(6, 128, N32)
    return pos16d, pos32d


def _scatter_indices(flat_keep, rank):
    """Destination row for every token in device stream order.

    flat_keep/rank: [PERCORE*246] bool / running-rank over this core's
    reference-ordered token list (cls + 196 + 49 per image).
    Returns [128, NSCAT] int32 (host-pretransposed for the SBUF layout).
    """
    idx_tiles = np.full((NSCAT, 128), TRASH, np.int32)
    t = 0

    def emit16(g, pi, t):
        dests = np.full(M16, TRASH, np.int64)
        k = 0
        for ig in range(GI):
            img = GI * g + ig
            for gy32 in range(G32):
                gy16 = 2 * gy32 + pi
                base = img * 246 + 1 + gy16 * G16
                for gx in range(G16):
                    fi = base + gx
                    dests[k] = rank[fi] if flat_keep[fi] else TRASH
                    k += 1
        for c0, cw in _CHUNKS16:
            idx_tiles[t, :cw] = dests[c0:c0 + cw]
            t += 1
        return t

    def emit32(g, t):
        dests = np.full(M32, TRASH, np.int64)
        k = 0
        for ig in range(GI):
            img = GI * g + ig
            base = img * 246 + 1 + N16
            for mm in range(N32):
                fi = base + mm
                dests[k] = rank[fi] if flat_keep[fi] else TRASH
                k += 1
        for c0, cw in _CHUNKS32:
            idx_tiles[t, :cw] = dests[c0:c0 + cw]
            t += 1
        return t

    for g in range(GROUPS):
        # must match device emission order: pi0 16-path, 32-path, pi1 16-path
        t = emit16(g, 0, t)
        t = emit32(g, t)
        t = emit16(g, 1, t)
    assert t == NSCAT
    return np.ascontiguousarray(idx_tiles.T)


# ---------------------------------------------------------------------------
# device program
# ---------------------------------------------------------------------------

def _build_program(bench=False):
    nc = bass.Bass()
    xin = nc.declare_dram_parameter("x", [PERCORE, C, IMG, IMG], F32, isOutput=False)
    w16in = nc.declare_dram_parameter("w16d", [8, 112, D], F32, isOutput=False)
    w32in = nc.declare_dram_parameter("w32d", [32, 112, D], F32, isOutput=False)
    p16in = nc.declare_dram_parameter("pos16d", [6, 128, 2, 98], F32, isOutput=False)
    p32in = nc.declare_dram_parameter("pos32d", [6, 128, N32], F32, isOutput=False)
    idxin = nc.declare_dram_parameter("idxd", [128, NSCAT], I32, isOutput=False)
    if bench:
        # timing variant: scatter goes to internal DRAM scratch so the
        # donated ExternalOutput zero-buffer is tiny
        outb = nc.dram_tensor("outb_scratch", [OUT_ROWS, D], F32)
        done = nc.declare_dram_parameter("done", [1, 4], F32, isOutput=True)
    else:
        outb = nc.declare_dram_parameter("outb", [OUT_ROWS, D], F32, isOutput=True)
        done = None

    with tile.TileContext(nc) as tc:
        with (
            tc.tile_pool(name="const", bufs=1) as cpool,
            tc.tile_pool(name="xp", bufs=1) as xpool,
            tc.tile_pool(name="x2p", bufs=1) as x2pool,
            tc.tile_pool(name="evp", bufs=1) as evpool,
            tc.tile_pool(name="tmp", bufs=2) as tmpool,
            tc.tile_pool(name="mmps", bufs=2, space="PSUM") as mmpool,
            tc.tile_pool(name="trps", bufs=3, space="PSUM") as trpool,
        ):
            w16sb = cpool.tile([112, 8 * D], F32, tag="w16")
            w32sb = cpool.tile([112, 32 * D], F32, tag="w32")
            # pos16 | pos32 | identity | idx (int32 view), one padded slot
            csb = cpool.tile([128, 1176 + 294 + 128 + NSCAT], F32, tag="cc")
            pos16sb = csb[:, 0:1176]
            pos32sb = csb[:, 1176:1470]
            ident = csb[:, 1470:1598]
            idxsb = csb[:, 1598:1598 + NSCAT].bitcast(I32)
            make_identity(nc, ident)

            nc.sync.dma_start(
                out=w16sb[:],
                in_=w16in[:].rearrange("j r d -> r j d"),
            )
            nc.sync.dma_start(
                out=w32sb[:],
                in_=w32in[:].rearrange("j r d -> r j d"),
            )
            nc.sync.dma_start(
                out=pos16sb,
                in_=p16in[:].rearrange("t p pi m -> p t pi m"),
            )
            nc.sync.dma_start(
                out=pos32sb,
                in_=p32in[:].rearrange("t p m -> p t m"),
            )
            nc.sync.dma_start(out=idxsb, in_=idxin[:])

            def scatter(tm_ap, cw, stream_tile):
                nc.gpsimd.indirect_dma_start(
                    out=outb[:],
                    out_offset=bass.IndirectOffsetOnAxis(
                        ap=idxsb[0:cw, stream_tile:stream_tile + 1], axis=0
                    ),
                    in_=tm_ap,
                    in_offset=None,
                )

            def transpose_and_scatter(ev6, mtot, chunks, stream_tile):
                for c0, cw in chunks:
                    tm = tmpool.tile([128, 6 * 128], F32, tag="tm")
                    for dt in range(6):
                        tr = trpool.tile([128, 128], F32, tag="tr")
                        nc.tensor.transpose(
                            out=tr[0:cw, :],
                            in_=ev6[:, dt * M16 + c0: dt * M16 + c0 + cw],
                            identity=ident,
                        )
                        nc.scalar.copy(
                            out=tm[0:cw, dt * 128:(dt + 1) * 128],
                            in_=tr[0:cw, :],
                        )
                    scatter(tm[0:cw, :], cw, stream_tile)
                    stream_tile += 1
                return stream_tile

            def path16(xsb, pi, stream_tile):
                x2 = x2pool.tile([128, XW], F32, tag="x2")
                nc.vector.tensor_copy(
                    out=x2[0:48, :], in_=xsb[64 * pi:64 * pi + 48, :]
                )
                nc.vector.tensor_copy(
                    out=x2[64:112, 0:XW - 8],
                    in_=xsb[64 * pi:64 * pi + 48, 8:XW],
                )
                x2r = x2[0:112].rearrange(
                    "p (ig gy gx j) -> p ig gy gx j", ig=GI, gy=G32, gx=G16
                )
                ev6 = evpool.tile([128, 6 * M16], F32, tag="ev")
                for dt in range(6):
                    ps = mmpool.tile([128, M16], F32, tag="mm")
                    for j in range(8):
                        nc.tensor.matmul(
                            out=ps[:],
                            lhsT=w16sb[:, j * D + dt * 128: j * D + dt * 128 + 128],
                            rhs=x2r[:, :, :, :, j],
                            start=(j == 0),
                            stop=(j == 7),
                        )
                    s0 = (dt * 2 + pi) * 98
                    pslice = pos16sb[:, s0:s0 + 98]
                    pbc = bass.AP(
                        pslice.tensor, pslice.offset,
                        [pslice.ap[0], [0, GI], [1, 98]],
                    )
                    nc.vector.tensor_tensor(
                        out=ev6[:, dt * M16:(dt + 1) * M16].rearrange(
                            "p (ig m) -> p ig m", ig=GI),
                        in0=ps[:].rearrange("p (ig m) -> p ig m", ig=GI),
                        in1=pbc,
                        op=mybir.AluOpType.add,
                    )
                return transpose_and_scatter(ev6, M16, _CHUNKS16, stream_tile)

            def path32(xsb, stream_tile):
                xr = xsb[0:112].rearrange(
                    "p (ig gy gx j) -> p ig gy gx j", ig=GI, gy=G32, gx=G32
                )
                ev6 = evpool.tile([128, 6 * M16], F32, tag="ev")
                for dt in range(6):
                    ps = mmpool.tile([128, M16], F32, tag="mm")
                    for j in range(32):
                        nc.tensor.matmul(
                            out=ps[:, 0:M32],
                            lhsT=w32sb[:, j * D + dt * 128: j * D + dt * 128 + 128],
                            rhs=xr[:, :, :, :, j],
                            start=(j == 0),
                            stop=(j == 31),
                        )
                    pslice = pos32sb[:, dt * N32:(dt + 1) * N32]
                    pbc = bass.AP(
                        pslice.tensor, pslice.offset,
                        [pslice.ap[0], [0, GI], [1, N32]],
                    )
                    nc.vector.tensor_tensor(
                        out=ev6[:, dt * M16: dt * M16 + M32].rearrange(
                            "p (ig m) -> p ig m", ig=GI),
                        in0=ps[:, 0:M32].rearrange("p (ig m) -> p ig m", ig=GI),
                        in1=pbc,
                        op=mybir.AluOpType.add,
                    )
                return transpose_and_scatter(ev6, M32, _CHUNKS32, stream_tile)

            # one-time zero of the never-written partition bands (48:64,
            # 112:128) of the single x / x2 slots so garbage*0 can't make NaN
            xz = xpool.tile([128, XW], F32, tag="x")
            x2z = x2pool.tile([128, XW], F32, tag="x2")
            for z in (xz, x2z):
                nc.gpsimd.memset(z[32:64, :], 0.0)
                nc.gpsimd.memset(z[96:128, :], 0.0)

            stream_tile = 0
            for g in range(GROUPS):
                xsb = xpool.tile([128, XW], F32, tag="x")
                for ig in range(GI):
                    img = GI * g + ig
                    imgr = xin[:][img].rearrange(
                        "c (gy pi i) w -> pi i c gy w", pi=2, i=16
                    )
                    for pi in range(2):
                        nc.sync.dma_start(
                            out=xsb[64 * pi:64 * pi + 48,
                                    ig * ROWFREE:(ig + 1) * ROWFREE],
                            in_=imgr[pi],
                        )
                # order: pi0 16-path, 32-path, pi1 16-path — so the pi1 x2
                # copy (WAR on pi0 matmuls) overlaps the 32-path matmuls,
                # and the next group's x DMA overlaps pi1 matmuls.
                stream_tile = path16(xsb, 0, stream_tile)
                stream_tile = path32(xsb, stream_tile)
                stream_tile = path16(xsb, 1, stream_tile)
            assert stream_tile == NSCAT
            if done is not None:
                nc.sync.dma_start(out=done[:], in_=csb[0:1, 0:4])

    _split_multi_waits(nc)
    return nc


_PROGRAM_CACHE = {}


def _get_program(bench=False):
    key = "bench" if bench else "nc"
    if key not in _PROGRAM_CACHE:
        _PROGRAM_CACHE[key] = _build_program(bench)
    return _PROGRAM_CACHE[key]


# ---------------------------------------------------------------------------
# public entry point
# ---------------------------------------------------------------------------

def kernel(x, entropy_maps, w16, b16, w32, b32, pos16, pos32, cls_token,
           _want_timing=False):
    x = np.ascontiguousarray(np.asarray(x), dtype=np.float32)
    entropy_maps = np.asarray(entropy_maps)
    w16 = np.asarray(w16, dtype=np.float32)
    b16 = np.asarray(b16, dtype=np.float32)
    w32 = np.asarray(w32, dtype=np.float32)
    b32 = np.asarray(b32, dtype=np.float32)
    pos16 = np.asarray(pos16, dtype=np.float32)
    pos32 = np.asarray(pos32, dtype=np.float32)
    cls_token = np.asarray(cls_token, dtype=np.float32)

    mask32, mask16 = _masks(np.asarray(entropy_maps, dtype=np.float32))
    m_all = np.concatenate(
        [np.ones((B, 1), bool), mask16.reshape(B, N16), mask32.reshape(B, N32)],
        axis=1,
    )  # [B, 246]
    seqlens = m_all.sum(axis=1).astype(np.int32)
    cls_token_indices = np.concatenate(
        [np.zeros(1, np.int64), np.cumsum(seqlens)[:-1]]
    ).astype(np.int32)

    W16d, W32d = _pack_weights(w16, w32)
    pos16d, pos32d = _pack_pos(pos16, b16, pos32, b32)

    in_maps = []
    counts = []
    for core in range(NCORES):
        sub = m_all[PERCORE * core: PERCORE * (core + 1)]
        flat = sub.reshape(-1)
        rank = np.cumsum(flat) - 1
        counts.append(int(flat.sum()))
        idxd = _scatter_indices(flat, rank)
        in_maps.append(dict(
            x=x[PERCORE * core: PERCORE * (core + 1)],
            w16d=W16d, w32d=W32d, pos16d=pos16d, pos32d=pos32d, idxd=idxd,
        ))

    nc = _get_program()
    kres = run_bass_kernel_spmd(
        nc, in_maps, list(range(NCORES)), trace=bool(_want_timing)
    )
    results = kres.results

    ntotal = int(seqlens.sum())
    out = np.empty((ntotal, D), np.float32)
    off = 0
    for core in range(NCORES):
        n = counts[core]
        out[off:off + n] = results[core]["outb"][:n]
        off += n
    assert off == ntotal
    out[cls_token_indices.astype(np.int64)] = cls_token[0, 0]

    ret = (out[None], seqlens, cls_token_indices)
    if _want_timing:
        return ret, kres
    return ret
